# revision 28
# baseline (speedup 1.0000x reference)
"""Trainium2 Bass kernel for the contrastive loss problem.

Math reformulation of the reference (no [N, 2N-1] scatter needed):
  lse_i = log( exp(pos_val_i) + sum_{j in neg} exp(S_ij) + (2N-2-num_neg_i) )
  loss  = mean_i (lse_i - pos_val_i)
with S = (cos + 1) * 0.25, cos from row-normalized embeddings.

Sharding uses the Gram matrix's symmetry: core c computes only the
[512, 512*5] strip of exp(S) pairing its rows with block-columns
{c, c+1, .., c+4} (mod 8). Columns are pre-rotated on the host so the
program is identical on every core (SPMD). A single masked array
esn = (y_col != y_row) * exp(S) serves both row sums (stt accum_out)
and, for the distance-1..3 foreign blocks, per-column sums via
ones-vector matmuls accumulated in PSUM; the host adds those to the
foreign rows' totals. Distance-4 blocks are computed by both endpoint
cores (row sums only). The matmul runs in fp8 e4m3 (DoubleRow, K=256
per op) on x16-prescaled unit rows, loop-ordered m->k2->j so one
weight load serves 5 column streams into 5 concurrent PSUM banks.

The contraction dim is staged as 4 separate k-pair tiles (not one
[KC,...] tile) so each weight load depends only on its own DMA, and
the input DMAs are spread across the sync/scalar/vector queues
(gpsimd's DMA ring has a multi-us drain cost at teardown).

Host: norms, fp8 casts, rotation, first-positive gather and its
<e_i, e_firstpos(i)> dot products (label metadata / O(N*D) prep),
final assembly of ~4096 scalars.
"""

import sys

sys.path.insert(0, "/opt/trn_rl_repo")

from contextlib import ExitStack

import ml_dtypes
import numpy as np

import concourse.bacc as bacc
import concourse.tile as tile
from concourse import mybir
from concourse.bass_utils import run_bass_kernel_spmd

N, D = 4096, 1024
NCORES = 8
R = N // NCORES            # 512 rows per core
P = 128                    # partitions
MI = R // P                # 4 row chunks per core
KC = D // P                # 8 contraction chunks
KP = KC // 2               # 4 DoubleRow k-pairs
JW = 512                   # j tile width (one PSUM bank)
NB = 5                     # block-columns per core (self + 4 right neighbors)
JCOLS = NB * JW            # 2560
EPS = 1e-8
BF16 = ml_dtypes.bfloat16
FP8 = ml_dtypes.float8_e4m3
SCALE = 16.0
NWARM = 9

_CACHE = {}


def _build_program():
    nc = bacc.Bacc("TRN2", target_bir_lowering=False, debug=False)
    f32, bf16, fp8 = mybir.dt.float32, mybir.dt.bfloat16, mybir.dt.float8e4
    AF = mybir.ActivationFunctionType
    OP = mybir.AluOpType

    et_d = nc.dram_tensor("et", [KP, P, 2, JCOLS], fp8, kind="ExternalInput")
    yt_d = nc.dram_tensor("yt", [P, JCOLS], bf16, kind="ExternalInput")
    yb_d = nc.dram_tensor("yb", [P, MI], f32, kind="ExternalInput")
    # single output tensor (one DMA; two trailing DMAs serialize their
    # ~2us HBM-write receipts): cols 0:20 row sums, 20:532 column sums
    out_d = nc.dram_tensor("out", [P, MI * NB + JW], f32,
                           kind="ExternalOutput")

    with tile.TileContext(nc) as tc, ExitStack() as ctx:
        sb = ctx.enter_context(tc.tile_pool(name="sb", bufs=1))
        psum = ctx.enter_context(tc.tile_pool(name="psum", bufs=1,
                                              space="PSUM"))

        etp = [sb.tile([P, 2, JCOLS], fp8, tag=f"etp{i}", name=f"etp{i}")
               for i in range(KP)]
        yt = sb.tile([P, JCOLS], bf16, tag="yt")
        yb = sb.tile([P, MI], f32, tag="yb")
        b025 = sb.tile([P, 1], f32, tag="b025")
        nc.vector.memset(b025, 0.25)
        ones = sb.tile([P, 1], bf16, tag="ones")
        nc.vector.memset(ones, 1.0)
        outt = sb.tile([P, MI * NB + JW], f32, tag="outt")
        tacc = outt[:, 0:MI * NB]
        csev = outt[:, MI * NB:MI * NB + JW]
        # PSUM is exactly 8 banks: 5 rotating accumulators (pt0-4) plus 3
        # extras (ept0-2) that let chunk 0 also compute chunk 1's j=0..2
        # during its DMA-paced phase. ept0 doubles as the column-sum
        # accumulator once exp(m=1,j=0) has drained it.
        epts = {j: psum.tile([P, JW], f32, tag=f"ept{j}", name=f"ept{j}")
                for j in range(3)}
        cs1 = epts[0]

        # Input DMAs: the per-core HBM share (~358 GB/s) paces the head,
        # so split every k-pair across both HWDGE rings so pairs complete
        # strictly in consumption order (~1.8us apart). Emitted before any
        # other scalar-queue work (the exp table load would delay the
        # descriptor issue by ~1.3us).
        H = JCOLS // 2
        for i in range(KP):
            nc.sync.dma_start(out=etp[i][:, :, 0:H], in_=et_d[i][:, :, 0:H])
            nc.scalar.dma_start(out=etp[i][:, :, H:JCOLS],
                                in_=et_d[i][:, :, H:JCOLS])
        nc.sync.dma_start(out=yt, in_=yt_d[:])
        nc.scalar.dma_start(out=yb, in_=yb_d[:])

        # pre-load the exp table during the initial DMA wait
        warm = sb.tile([P, 1], f32, tag="warm")
        nc.scalar.activation(warm, b025, AF.Exp, bias=b025, scale=1.0)
        # keep the PE clock gate warm (and ramp its p-state) during the
        # DMA wait: full-width matmuls into a partition strip the column
        # sums never touch
        wsrc = sb.tile([P, JW], bf16, tag="wsrc")
        nc.vector.memset(wsrc, 1.0)

        def warm_mm(n):
            for _ in range(n):
                nc.tensor.matmul(
                    epts[1][96:97, :], ones, wsrc, start=True, stop=True,
                    tile_position=(0, 96), skip_group_check=True,
                )

        warm_mm(NWARM)

        esn_keep = {}

        def emit_cs(m):
            # column sums of the masked exp for the foreign blocks
            # (distance 1..3), accumulated over the 4 row chunks at
            # partition 32*jj of the reused ept0 PSUM bank
            for jj in range(3):
                pb = 32 * jj
                nc.tensor.matmul(
                    cs1[pb:pb + 1, :], ones, esn_keep[(m, jj)],
                    start=(m == 0), stop=(m == MI - 1),
                    tile_position=(0, pb), skip_group_check=True,
                )

        def emit_exp(m, j, pt):
            # expS = exp(cos*0.25 + 0.25)
            es = sb.tile([P, JW], bf16, tag="es", bufs=8, name="es")
            nc.scalar.activation(
                es, pt, AF.Exp, bias=b025, scale=0.25 / (SCALE * SCALE))
            # esn = (y != y_row) * expS; row sums via accum_out
            esn = sb.tile([P, JW], bf16, tag="esn", bufs=8, name="esn")
            nc.vector.scalar_tensor_tensor(
                esn, yt[:, j * JW:(j + 1) * JW], yb[:, m:m + 1], es,
                op0=OP.not_equal, op1=OP.mult,
                accum_out=tacc[:, m * NB + j:m * NB + j + 1],
            )
            if 1 <= j <= 3:
                esn_keep[(m, j - 1)] = esn

        PRE = [0, 1, 2]    # chunk 1 streams computed during chunk 0
        for m in range(MI):
            # for the last chunk, handle the blocks that feed column sums
            # first so the trailing CS matmuls aren't waiting on them
            jorder = [1, 2, 3, 0, 4] if m == MI - 1 else list(range(NB))
            if m == 1:
                # drain the precomputed accumulators first (the column-sum
                # matmuls below clobber ept0's partitions 0/32/64)
                for j in PRE:
                    emit_exp(1, j, epts[j])
            stream_js = [j for j in jorder
                         if not (m == 1 and j in PRE)]
            pts = {j: psum.tile([P, JW], f32, tag=f"pt{j}", name=f"pt{j}")
                   for j in stream_js}
            for k2 in range(KP):
                for j in stream_js:
                    nc.tensor.matmul(
                        pts[j],
                        etp[k2][:, :, m * P:(m + 1) * P],
                        etp[k2][:, :, j * JW:(j + 1) * JW],
                        start=(k2 == 0),
                        stop=(k2 == KP - 1),
                        perf_mode=mybir.MatmulPerfMode.DoubleRow,
                    )
                if m == 0:
                    # chunk 0 is DMA-paced (~1.8us per k-pair vs ~1.1us of
                    # streams): pack the wait with chunk 1's first blocks
                    for j in PRE:
                        nc.tensor.matmul(
                            epts[j],
                            etp[k2][:, :, 1 * P:2 * P],
                            etp[k2][:, :, j * JW:(j + 1) * JW],
                            start=(k2 == 0),
                            stop=(k2 == KP - 1),
                            perf_mode=mybir.MatmulPerfMode.DoubleRow,
                        )

                # the previous chunk's masked tiles are ready by k2==2;
                # this slot keeps the column sums off the critical tail
                if k2 == 2 and m >= 2:
                    emit_cs(m - 1)
            if m == 1:
                emit_cs(0)
            for j in (stream_js if m == 1 else jorder):
                emit_exp(m, j, pts[j])
        emit_cs(MI - 1)
        # evict column sums (DMA cannot read PSUM), then one output DMA
        nc.scalar.copy(csev[0:96, :], cs1[0:96, :])
        nc.sync.dma_start(out=out_d[:, :], in_=outt)

    nc.compile()
    return nc


def _get_program():
    if "nc" not in _CACHE:
        _CACHE["nc"] = _build_program()
    return _CACHE["nc"]


def _host_prep(layer_embeds, y_true):
    E = np.asarray(layer_embeds, dtype=np.float32)
    y = np.asarray(y_true).astype(np.int32)

    norms = np.maximum(np.linalg.norm(E, axis=1), EPS).astype(np.float32)
    Ehf = E / norms[:, None]
    Eh8T = np.ascontiguousarray((Ehf * SCALE).astype(FP8).T)  # [D, N]

    same = y[:, None] == y[None, :]
    nsame = same.sum(1)
    haspos = nsame > 1
    np.fill_diagonal(same, False)
    fp = np.argmax(same, axis=1)                      # first positive (j order)
    posd = np.einsum("ij,ij->i", Ehf.astype(np.float64),
                     Ehf[fp].astype(np.float64))      # <e_i, e_firstpos(i)>
    yb16 = y.astype(BF16)

    in_maps = []
    for c in range(NCORES):
        r0, r1 = c * R, (c + 1) * R
        cols = np.concatenate(
            [np.arange(((c + b) % NCORES) * R, ((c + b) % NCORES) * R + R)
             for b in range(NB)])
        # [KP, P, 2, JCOLS]: pair i row r partition p = D-row (2i+r)*128+p
        etc = np.ascontiguousarray(
            Eh8T[:, cols].reshape(KP, 2, P, JCOLS).transpose(0, 2, 1, 3))
        ytc = np.ascontiguousarray(
            np.broadcast_to(yb16[cols][None, :], (P, JCOLS)))
        in_maps.append({
            "et": etc,
            "yt": ytc,
            "yb": np.ascontiguousarray(y[r0:r1].astype(np.float32)
                                       .reshape(MI, P).T),
        })
    meta = {"haspos": haspos, "nsame": nsame, "posd": posd}
    return in_maps, meta


def _assemble(results, meta):
    """Combine per-core partials into the scalar loss (O(N) host math)."""
    haspos = meta["haspos"]
    nsame = meta["nsame"]
    posd = meta["posd"]

    neg = np.zeros(N, dtype=np.float64)   # per-row negative exp sums
    for c in range(NCORES):
        out = np.asarray(results[c]["out"], np.float64)  # [P, MI*NB + JW]
        rows = np.arange(c * R, (c + 1) * R)
        ro = out[:, 0:MI * NB]                           # [P, MI*NB]
        neg[rows] += ro.reshape(P, MI, NB).sum(2).T.reshape(-1)
        cs = out[:, MI * NB:]                            # [P, JW]
        for d in range(1, 4):
            b = (c + d) % NCORES
            rows_b = np.arange(b * R, b * R + R)
            # partition 32*(d-1) holds the [1, 512] column sums of the
            # distance-d block; JW == R so they map 1:1 onto b's rows
            neg[rows_b] += cs[32 * (d - 1), :]

    posS = (posd + 1.0) * 0.25
    nneg = N - nsame
    total = neg + np.where(haspos, np.exp(posS), 1.0) + (2 * N - 2 - nneg)
    posval = np.where(haspos, posS, 0.0)
    loss = float(np.mean(np.log(total) - posval))
    return np.float32(loss)


def _install_ntff_shim():
    """Provide antenv.axon_hooks (absent in this image) so trace=True works."""
    import importlib
    import types
    try:
        importlib.import_module("antenv.axon_hooks")
        return
    except ImportError:
        pass
    try:
        import antenv
        from trn_agent_boot.trn_boot import _ntff_profile_via_ctypes

        hook = _ntff_profile_via_ctypes("/opt/axon/libaxon_pjrt.so")
        mod = types.ModuleType("antenv.axon_hooks")
        mod._hook = hook
        mod.get_axon_ntff_profile_hook = lambda: mod._hook
        mod.set_axon_ntff_profile_hook = lambda h: setattr(mod, "_hook", h)
        sys.modules["antenv.axon_hooks"] = mod
        antenv.axon_hooks = mod
    except Exception as e:  # profiling is best-effort
        print(f"ntff shim failed: {e}")


def kernel(layer_embeds, y_true, _trace=False):
    import time

    if _trace:
        _install_ntff_shim()
    nc = _get_program()
    in_maps, meta = _host_prep(layer_embeds, y_true)
    last_err = None
    for attempt in range(4):
        try:
            res = run_bass_kernel_spmd(
                nc, in_maps, core_ids=list(range(NCORES)), trace=_trace,
            )
            loss = _assemble(res.results, meta)
            # lse is bounded by log(2N-2) .. log(2N + N*e^0.5) for this
            # problem shape; anything outside is transient corruption.
            if not (np.isfinite(loss) and 5.0 < float(loss) < 20.0):
                raise RuntimeError(f"implausible loss {loss}, retrying")
            if _trace:
                return loss, res
            return loss
        except Exception as e:  # transient device faults: retry
            last_err = e
            time.sleep(5 * (attempt + 1))
    raise last_err


# revision 30
# speedup vs baseline: 1.0104x; 1.0104x over previous
"""Trainium2 Bass kernel for the contrastive loss problem.

Math reformulation of the reference (no [N, 2N-1] scatter needed):
  lse_i = log( exp(pos_val_i) + sum_{j in neg} exp(S_ij) + (2N-2-num_neg_i) )
  loss  = mean_i (lse_i - pos_val_i)
with S = (cos + 1) * 0.25, cos from row-normalized embeddings.

Sharding uses the Gram matrix's symmetry: core c computes only the
[512, 512*5] strip of exp(S) pairing its rows with block-columns
{c, c+1, .., c+4} (mod 8). Columns are pre-rotated on the host so the
program is identical on every core (SPMD). A single masked array
esn = (y_col != y_row) * exp(S) serves both row sums (stt accum_out)
and, for the distance-1..3 foreign blocks, per-column sums via
ones-vector matmuls accumulated in PSUM; the host adds those to the
foreign rows' totals. Distance-4 blocks are computed by both endpoint
cores (row sums only). The matmul runs in fp8 e4m3 (DoubleRow, K=256
per op) on x16-prescaled unit rows, loop-ordered m->k2->j so one
weight load serves 5 column streams into 5 concurrent PSUM banks.

The contraction dim is staged as 4 separate k-pair tiles (not one
[KC,...] tile) so each weight load depends only on its own DMA, and
the input DMAs are spread across the sync/scalar/vector queues
(gpsimd's DMA ring has a multi-us drain cost at teardown).

Host: norms, fp8 casts, rotation, first-positive gather and its
<e_i, e_firstpos(i)> dot products (label metadata / O(N*D) prep),
final assembly of ~4096 scalars.
"""

import sys

sys.path.insert(0, "/opt/trn_rl_repo")

from contextlib import ExitStack

import ml_dtypes
import numpy as np

import concourse.bacc as bacc
import concourse.tile as tile
from concourse import mybir
from concourse.bass_utils import run_bass_kernel_spmd

N, D = 4096, 1024
NCORES = 8
R = N // NCORES            # 512 rows per core
P = 128                    # partitions
MI = R // P                # 4 row chunks per core
KC = D // P                # 8 contraction chunks
KP = KC // 2               # 4 DoubleRow k-pairs
JW = 512                   # j tile width (one PSUM bank)
NB = 5                     # block-columns per core (self + 4 right neighbors)
JCOLS = NB * JW            # 2560
EPS = 1e-8
BF16 = ml_dtypes.bfloat16
FP8 = ml_dtypes.float8_e4m3
SCALE = 16.0
NWARM = 9

_CACHE = {}


def _build_program():
    nc = bacc.Bacc("TRN2", target_bir_lowering=False, debug=False)
    f32, bf16, fp8 = mybir.dt.float32, mybir.dt.bfloat16, mybir.dt.float8e4
    AF = mybir.ActivationFunctionType
    OP = mybir.AluOpType

    et_d = nc.dram_tensor("et", [KP, P, 2, JCOLS], fp8, kind="ExternalInput")
    yt_d = nc.dram_tensor("yt", [P, JCOLS], bf16, kind="ExternalInput")
    yb_d = nc.dram_tensor("yb", [P, MI], f32, kind="ExternalInput")
    # single output tensor (one DMA; two trailing DMAs serialize their
    # ~2us HBM-write receipts): cols 0:20 row sums, 20:532 column sums
    out_d = nc.dram_tensor("out", [P, MI * NB + JW], f32,
                           kind="ExternalOutput")

    with tile.TileContext(nc) as tc, ExitStack() as ctx:
        sb = ctx.enter_context(tc.tile_pool(name="sb", bufs=1))
        psum = ctx.enter_context(tc.tile_pool(name="psum", bufs=1,
                                              space="PSUM"))

        etp = [sb.tile([P, 2, JCOLS], fp8, tag=f"etp{i}", name=f"etp{i}")
               for i in range(KP)]
        yt = sb.tile([P, JCOLS], bf16, tag="yt")
        yb = sb.tile([P, MI], f32, tag="yb")
        b025 = sb.tile([P, 1], f32, tag="b025")
        nc.vector.memset(b025, 0.25)
        ones = sb.tile([P, 1], bf16, tag="ones")
        nc.vector.memset(ones, 1.0)
        outt = sb.tile([P, MI * NB + JW], f32, tag="outt")
        tacc = outt[:, 0:MI * NB]
        csev = outt[:, MI * NB:MI * NB + JW]
        # PSUM is exactly 8 banks: 5 rotating accumulators (pt0-4) plus 3
        # extras (ept0-2) that let chunk 0 also compute chunk 1's j=0..2
        # during its DMA-paced phase. ept0 doubles as the column-sum
        # accumulator once exp(m=1,j=0) has drained it.
        epts = {j: psum.tile([P, JW], f32, tag=f"ept{j}", name=f"ept{j}")
                for j in range(3)}
        cs1 = epts[0]

        # Input DMAs: the per-core HBM share (~358 GB/s) paces the head,
        # so split every k-pair across both HWDGE rings so pairs complete
        # strictly in consumption order (~1.8us apart). Emitted before any
        # other scalar-queue work (the exp table load would delay the
        # descriptor issue by ~1.3us).
        H = JCOLS // 2
        for i in range(KP):
            nc.sync.dma_start(out=etp[i][:, :, 0:H], in_=et_d[i][:, :, 0:H])
            nc.scalar.dma_start(out=etp[i][:, :, H:JCOLS],
                                in_=et_d[i][:, :, H:JCOLS])
        nc.sync.dma_start(out=yt, in_=yt_d[:])
        nc.scalar.dma_start(out=yb, in_=yb_d[:])

        # pre-load the exp table during the initial DMA wait
        warm = sb.tile([P, 1], f32, tag="warm")
        nc.scalar.activation(warm, b025, AF.Exp, bias=b025, scale=1.0)
        # keep the PE clock gate warm (and ramp its p-state) during the
        # DMA wait: full-width matmuls into a partition strip the column
        # sums never touch
        wsrc = sb.tile([P, JW], bf16, tag="wsrc")
        nc.vector.memset(wsrc, 1.0)

        def warm_mm(n):
            for _ in range(n):
                nc.tensor.matmul(
                    epts[1][96:97, :], ones, wsrc, start=True, stop=True,
                    tile_position=(0, 96), skip_group_check=True,
                )

        warm_mm(NWARM)

        esn_keep = {}

        def emit_cs(m):
            # column sums of the masked exp for the foreign blocks
            # (distance 1..3), accumulated over the 4 row chunks at
            # partition 32*jj of the reused ept0 PSUM bank
            for jj in range(3):
                pb = 32 * jj
                nc.tensor.matmul(
                    cs1[pb:pb + 1, :], ones, esn_keep[(m, jj)],
                    start=(m == 0), stop=(m == MI - 1),
                    tile_position=(0, pb), skip_group_check=True,
                )

        def emit_exp(m, j, pt):
            # expS = exp(cos*0.25 + 0.25)
            es = sb.tile([P, JW], bf16, tag="es", bufs=8, name="es")
            nc.scalar.activation(
                es, pt, AF.Exp, bias=b025, scale=0.25 / (SCALE * SCALE))
            # esn = (y != y_row) * expS; row sums via accum_out
            esn = sb.tile([P, JW], bf16, tag="esn", bufs=8, name="esn")
            nc.vector.scalar_tensor_tensor(
                esn, yt[:, j * JW:(j + 1) * JW], yb[:, m:m + 1], es,
                op0=OP.not_equal, op1=OP.mult,
                accum_out=tacc[:, m * NB + j:m * NB + j + 1],
            )
            if 1 <= j <= 3:
                esn_keep[(m, j - 1)] = esn

        PRE = [0, 1, 2]    # chunk 1 streams computed during chunk 0
        for m in range(MI):
            # for the last chunk, handle the blocks that feed column sums
            # first so the trailing CS matmuls aren't waiting on them
            jorder = [1, 2, 3, 0, 4] if m == MI - 1 else list(range(NB))
            if m == 1:
                # drain the precomputed accumulators first (the column-sum
                # matmuls below clobber ept0's partitions 0/32/64)
                for j in PRE:
                    emit_exp(1, j, epts[j])
            stream_js = [j for j in jorder
                         if not (m == 1 and j in PRE)]
            pts = {j: psum.tile([P, JW], f32, tag=f"pt{j}", name=f"pt{j}")
                   for j in stream_js}
            for k2 in range(KP):
                for j in stream_js:
                    nc.tensor.matmul(
                        pts[j],
                        etp[k2][:, :, m * P:(m + 1) * P],
                        etp[k2][:, :, j * JW:(j + 1) * JW],
                        start=(k2 == 0),
                        stop=(k2 == KP - 1),
                        perf_mode=mybir.MatmulPerfMode.DoubleRow,
                    )
                if m == 0:
                    # chunk 0 is DMA-paced (~1.8us per k-pair vs ~1.1us of
                    # streams): pack the wait with chunk 1's first blocks
                    for j in PRE:
                        nc.tensor.matmul(
                            epts[j],
                            etp[k2][:, :, 1 * P:2 * P],
                            etp[k2][:, :, j * JW:(j + 1) * JW],
                            start=(k2 == 0),
                            stop=(k2 == KP - 1),
                            perf_mode=mybir.MatmulPerfMode.DoubleRow,
                        )

                # the previous chunk's masked tiles are ready by k2==2;
                # this slot keeps the column sums off the critical tail
                if k2 == 2 and m >= 2:
                    emit_cs(m - 1)
            if m == 1:
                emit_cs(0)
            for j in (stream_js if m == 1 else jorder):
                emit_exp(m, j, pts[j])
        emit_cs(MI - 1)
        # evict column sums (DMA cannot read PSUM), then one output DMA
        nc.scalar.copy(csev[0:96, :], cs1[0:96, :])
        nc.sync.dma_start(out=out_d[:, :], in_=outt)

    nc.compile()
    return nc


def _get_program():
    if "nc" not in _CACHE:
        _CACHE["nc"] = _build_program()
    return _CACHE["nc"]


def _host_prep(layer_embeds, y_true):
    E = np.asarray(layer_embeds, dtype=np.float32)
    y = np.asarray(y_true).astype(np.int32)

    norms = np.maximum(np.linalg.norm(E, axis=1), EPS).astype(np.float32)
    Ehf = E / norms[:, None]
    Eh8T = np.ascontiguousarray((Ehf * SCALE).astype(FP8).T)  # [D, N]

    same = y[:, None] == y[None, :]
    nsame = same.sum(1)
    haspos = nsame > 1
    np.fill_diagonal(same, False)
    fp = np.argmax(same, axis=1)                      # first positive (j order)
    posd = np.einsum("ij,ij->i", Ehf.astype(np.float64),
                     Ehf[fp].astype(np.float64))      # <e_i, e_firstpos(i)>
    yb16 = y.astype(BF16)

    in_maps = []
    for c in range(NCORES):
        r0, r1 = c * R, (c + 1) * R
        cols = np.concatenate(
            [np.arange(((c + b) % NCORES) * R, ((c + b) % NCORES) * R + R)
             for b in range(NB)])
        # [KP, P, 2, JCOLS]: pair i row r partition p = D-row (2i+r)*128+p
        etc = np.ascontiguousarray(
            Eh8T[:, cols].reshape(KP, 2, P, JCOLS).transpose(0, 2, 1, 3))
        ytc = np.ascontiguousarray(
            np.broadcast_to(yb16[cols][None, :], (P, JCOLS)))
        in_maps.append({
            "et": etc,
            "yt": ytc,
            "yb": np.ascontiguousarray(y[r0:r1].astype(np.float32)
                                       .reshape(MI, P).T),
        })
    meta = {"haspos": haspos, "nsame": nsame, "posd": posd}
    return in_maps, meta


def _assemble(results, meta):
    """Combine per-core partials into the scalar loss (O(N) host math)."""
    haspos = meta["haspos"]
    nsame = meta["nsame"]
    posd = meta["posd"]

    neg = np.zeros(N, dtype=np.float64)   # per-row negative exp sums
    for c in range(NCORES):
        out = np.asarray(results[c]["out"], np.float64)  # [P, MI*NB + JW]
        rows = np.arange(c * R, (c + 1) * R)
        ro = out[:, 0:MI * NB]                           # [P, MI*NB]
        neg[rows] += ro.reshape(P, MI, NB).sum(2).T.reshape(-1)
        cs = out[:, MI * NB:]                            # [P, JW]
        for d in range(1, 4):
            b = (c + d) % NCORES
            rows_b = np.arange(b * R, b * R + R)
            # partition 32*(d-1) holds the [1, 512] column sums of the
            # distance-d block; JW == R so they map 1:1 onto b's rows
            neg[rows_b] += cs[32 * (d - 1), :]

    posS = (posd + 1.0) * 0.25
    nneg = N - nsame
    total = neg + np.where(haspos, np.exp(posS), 1.0) + (2 * N - 2 - nneg)
    posval = np.where(haspos, posS, 0.0)
    loss = float(np.mean(np.log(total) - posval))
    return np.float32(loss)


def _install_ntff_shim():
    """Provide antenv.axon_hooks (absent in this image) so trace=True works."""
    import importlib
    import types
    try:
        importlib.import_module("antenv.axon_hooks")
        return
    except ImportError:
        pass
    try:
        import antenv
        from trn_agent_boot.trn_boot import _ntff_profile_via_ctypes

        hook = _ntff_profile_via_ctypes("/opt/axon/libaxon_pjrt.so")
        mod = types.ModuleType("antenv.axon_hooks")
        mod._hook = hook
        mod.get_axon_ntff_profile_hook = lambda: mod._hook
        mod.set_axon_ntff_profile_hook = lambda h: setattr(mod, "_hook", h)
        sys.modules["antenv.axon_hooks"] = mod
        antenv.axon_hooks = mod
    except Exception as e:  # profiling is best-effort
        print(f"ntff shim failed: {e}")


def kernel(layer_embeds, y_true, _trace=False):
    import time

    if _trace:
        _install_ntff_shim()
    nc = _get_program()
    in_maps, meta = _host_prep(layer_embeds, y_true)
    last_err = None
    for attempt in range(4):
        try:
            res = run_bass_kernel_spmd(
                nc, in_maps, core_ids=list(range(NCORES)), trace=_trace,
            )
            loss = _assemble(res.results, meta)
            # lse is bounded by log(2N-2) .. log(2N + N*e^0.5) for this
            # problem shape; anything outside is transient corruption.
            if not (np.isfinite(loss) and 5.0 < float(loss) < 20.0):
                raise RuntimeError(f"implausible loss {loss}, retrying")
            if _trace:
                return loss, res
            return loss
        except Exception as e:  # transient device faults: retry
            last_err = e
            time.sleep(5 * (attempt + 1))
    raise last_err
